# revision 53
# baseline (speedup 1.0000x reference)
"""Multi-head causal attention (B=2, S=2048, D=1024, H=16) on 8 TRN2 NeuronCores.

Sharding: batch x head-group tensor parallel. Core c owns batch c//4 and
heads [4*(c%4), 4*(c%4)+4) (a DM=256 model-dim shard). The host
pre-transposes X per batch to XT [D, S] and casts all device inputs to
bf16; each core computes a partial output [S, D] for its batch; the host
sums the 4 partials per batch and adds bo.

Per-core program (all matmul operands bf16, PSUM accumulate f32):
  QT/KT = W^T XT    [256, 2048], m on partitions (2 groups of 128)
  V     = XT^T Wv   [2048, 256] natural (s on partitions), stored per
          head with an appended ones column (softmax denominator trick)
  attention per q-chunk (512) per head pair: scoresT = K Q^T on PE,
  exp on ACT (scale=1/8, both heads of the pair in one activation off a
  2-bank psum tile), causal diagonal-block mask applied multiplicatively
  post-exp on DVE (triangular constant, bf16 4x mode), PV with the ones
  column -> ctxT[64] + denominator row, reciprocal + gpsimd
  partition_broadcast -> normalized ctxT in bf16.
  out[s, :] = ctxT^T Wo (2 dm-chunks accumulated), f16 copy, DMA out.

Emission interleaves projection / out-projection psum groups into the
attention ki-loops (skew-1 scores->PV) so the in-order PE queue never
stalls on the exp->PV dependency.
"""

import numpy as np

B, S, D = 2, 2048, 1024
H_PER_CORE = 4
HD = 64
DM = H_PER_CORE * HD   # 256, per-core model-dim shard
N_CORES = 8
P = 128
QT_TILE = 512          # q free-dim tile in attention
KO = D // P            # 8 contraction chunks for projections
NSI = S // P           # 16 s-chunks of 128
NSJ = S // QT_TILE     # 4 s-chunks of 512

_BUILD_CACHE = {}


def build_bass(mm_mode: str = "bf16"):
    """Build the per-core Bass program. mm_mode in {bf16, fp32r}."""
    import contextlib

    import concourse.tile as tile
    from concourse import bacc, mybir
    from concourse.masks import make_upper_triangular

    f32 = mybir.dt.float32
    f16 = mybir.dt.float16
    DT = mybir.dt.bfloat16 if mm_mode == "bf16" else mybir.dt.float32r
    Exp = mybir.ActivationFunctionType.Exp
    mult_op = mybir.AluOpType.mult

    nc = bacc.Bacc("TRN2", target_bir_lowering=False, debug=False)

    XTd = nc.dram_tensor("XT", [D, S], DT, kind="ExternalInput").ap()
    Wq = nc.dram_tensor("Wq", [D, DM], DT, kind="ExternalInput").ap()
    Wk = nc.dram_tensor("Wk", [D, DM], DT, kind="ExternalInput").ap()
    Wv = nc.dram_tensor("Wv", [D, DM], DT, kind="ExternalInput").ap()
    Wo = nc.dram_tensor("Wo", [DM, D], DT, kind="ExternalInput").ap()
    Out = nc.dram_tensor("Out", [S, D], f16, kind="ExternalOutput").ap()

    lp_ctx = (nc.allow_low_precision(reason="bf16 rounding is intentional")
              if hasattr(nc, "allow_low_precision") else contextlib.nullcontext())
    with lp_ctx, tile.TileContext(nc) as tc:
        with tc.tile_pool(name="consts", bufs=1) as consts, \
             tc.tile_pool(name="wpool", bufs=1) as wpool, \
             tc.tile_pool(name="qt", bufs=2) as qtp, \
             tc.tile_pool(name="et", bufs=8) as etp, \
             tc.tile_pool(name="ctx", bufs=2) as ctxp, \
             tc.tile_pool(name="raw", bufs=4) as rawp, \
             tc.tile_pool(name="den", bufs=4) as denp, \
             tc.tile_pool(name="rbc", bufs=4) as rbcp, \
             tc.tile_pool(name="outp", bufs=8) as outp, \
             tc.tile_pool(name="ps_mm", bufs=2, space="PSUM") as ps_mm, \
             tc.tile_pool(name="ps_s", bufs=2, space="PSUM") as ps_s, \
             tc.tile_pool(name="ps_ctx", bufs=2, space="PSUM") as ps_ctx:

            # ---- constants ----
            # tri[k, q] = 1 where k <= q else 0 (keep-mask for the causal
            # diagonal 128x128 block of scoresT)
            tri = consts.tile([P, P], DT, tag="tri")
            make_upper_triangular(nc, tri[:], val=1.0, diag=True)

            # ---- persistent sbuf tensors ----
            xt = wpool.tile([P, KO, S], DT, tag="xt")
            wq_sb = wpool.tile([P, KO, DM], DT, tag="wq")
            wk_sb = wpool.tile([P, KO, DM], DT, tag="wk")
            wv_sb = wpool.tile([P, KO, DM], DT, tag="wv")
            wo_sb = wpool.tile([P, 2, D], DT, tag="wo")
            kt = wpool.tile([P, 2, S], DT, tag="kt")
            v = wpool.tile([P, NSI, H_PER_CORE, HD + 1], DT, tag="v")

            # warm the Act exp table during the DMA head so the load does
            # not stall the first attention chain
            warm = consts.tile([1, 1], DT, tag="warm")
            nc.scalar.activation(warm[:], tri[0:1, 0:1], Exp, scale=1.0)

            # warm-up matmuls: keep PE busy through the DMA head so the
            # p-state ramp is fully paid on junk work and the first real
            # matmuls run at peak clock
            wps = ps_s.tile([P, 2, QT_TILE], f32, tag="s", name="ps_warm")
            for _ in range(56):
                nc.tensor.matmul(wps[:, 0, 0:P], tri[:], tri[:],
                                 start=True, stop=True)

            # ---- input DMAs (HWDGE), ordered by first use; transfers are
            # effectively serialized on the DMA engines, so order = need ----
            nc.sync.dma_start(
                wv_sb[:, 0:4, :],
                Wv[0:4 * P, :].rearrange("(ko p) m -> p ko m", p=P))
            nc.scalar.dma_start(
                xt[:, :, 0:256],
                XTd[:, 0:256].rearrange("(ko p) s -> p ko s", p=P))
            nc.sync.dma_start(
                wv_sb[:, 4:8, :],
                Wv[4 * P:8 * P, :].rearrange("(ko p) m -> p ko m", p=P))
            nc.scalar.dma_start(
                xt[:, :, 256:QT_TILE],
                XTd[:, 256:QT_TILE].rearrange("(ko p) s -> p ko s", p=P))
            nc.sync.dma_start(wq_sb[:], Wq.rearrange("(ko p) m -> p ko m", p=P))
            nc.sync.dma_start(wk_sb[:], Wk.rearrange("(ko p) m -> p ko m", p=P))
            xt_sl = [slice(qj * QT_TILE, (qj + 1) * QT_TILE)
                     for qj in range(1, NSJ)]
            nc.scalar.dma_start(
                xt[:, :, xt_sl[0]],
                XTd[:, xt_sl[0]].rearrange("(ko p) s -> p ko s", p=P))
            nc.sync.dma_start(wo_sb[:], Wo.rearrange("(g p) n -> p g n", p=P))
            for sl in xt_sl[1:]:
                nc.scalar.dma_start(
                    xt[:, :, sl], XTd[:, sl].rearrange("(ko p) s -> p ko s", p=P)
                )

            # ones columns of V (denominator accumulator rows)
            nc.vector.memset(v[:, :, :, HD:HD + 1], 1.0)

            QT_SB = {}   # qj -> qt tile
            CTX_SB = {}  # qj -> ctx sbuf tile

            # ---- psum-group "units" (interleave fillers) ----
            def qk_unit(qj, g, w_sb, dst_ap):
                ps = ps_mm.tile([P, QT_TILE], f32, tag="mm", name="ps_qk")
                sl = slice(qj * QT_TILE, (qj + 1) * QT_TILE)
                for ko in range(KO):
                    nc.tensor.matmul(
                        ps[:], w_sb[:, ko, g * P:(g + 1) * P], xt[:, ko, sl],
                        start=(ko == 0), stop=(ko == KO - 1),
                    )
                nc.vector.tensor_copy(out=dst_ap, in_=ps[:])

            def v_unit(si):
                ps = ps_mm.tile([P, DM], f32, tag="mm", name="ps_v")
                for ko in range(KO):
                    nc.tensor.matmul(
                        ps[:], xt[:, ko, si * P:(si + 1) * P], wv_sb[:, ko, :],
                        start=(ko == 0), stop=(ko == KO - 1),
                    )
                nc.vector.tensor_copy(
                    out=v[:, si, :, 0:HD],
                    in_=ps[:].rearrange("p (h d) -> p h d", d=HD),
                )

            def proj_units(qj):
                qt_sb = qtp.tile([P, 2, QT_TILE], DT, tag="qt", name=f"qt{qj}")
                QT_SB[qj] = qt_sb
                units = []
                for g in range(2):
                    units.append(lambda g=g: qk_unit(qj, g, wq_sb, qt_sb[:, g, :]))
                for g in range(2):
                    units.append(lambda g=g: qk_unit(
                        qj, g, wk_sb,
                        kt[:, g, qj * QT_TILE:(qj + 1) * QT_TILE]))
                for si in range(4 * qj, 4 * qj + 4):
                    units.append(lambda si=si: v_unit(si))
                return units

            def oproj_unit(qj, si, dj, split_copy=False, c0=0, cw=QT_TILE,
                           on_act=False):
                ps = ps_mm.tile([P, QT_TILE], f32, tag="mm", name="ps_o")
                sc = (si % 4) * P
                n0 = dj * QT_TILE + c0
                for g in range(2):
                    nc.tensor.matmul(
                        ps[:, 0:cw], CTX_SB[qj][:, g, sc:sc + P],
                        wo_sb[:, g, n0:n0 + cw],
                        start=(g == 0), stop=(g == 1),
                    )
                ot = outp.tile([P, QT_TILE], f16, tag="ot", name="ot")
                if split_copy:
                    # tail-only: drain each psum group with both engines at
                    # once so the two mm-slot round trip stays short, and
                    # issue the two half stores on separate queues
                    h = cw // 2
                    nc.vector.tensor_copy(out=ot[:, 0:h], in_=ps[:, 0:h])
                    nc.scalar.copy(out=ot[:, h:cw], in_=ps[:, h:cw])
                    nc.sync.dma_start(
                        Out[si * P:(si + 1) * P, n0:n0 + h], ot[:, 0:h])
                    nc.scalar.dma_start(
                        Out[si * P:(si + 1) * P, n0 + h:n0 + cw],
                        ot[:, h:cw])
                    return
                if on_act:
                    nc.scalar.copy(out=ot[:, 0:cw], in_=ps[:, 0:cw])
                else:
                    nc.vector.tensor_copy(out=ot[:, 0:cw], in_=ps[:, 0:cw])
                nc.sync.dma_start(
                    Out[si * P:(si + 1) * P, n0:n0 + cw],
                    ot[:, 0:cw],
                )

            def oproj_units(qj, alternate=False):
                return [
                    lambda si=si, dj=dj: oproj_unit(
                        qj, si, dj, split_copy=alternate)
                    for si in range(4 * qj, 4 * qj + 4) for dj in range(2)
                ]

            LAST_RAW = {"ap": None}

            def junk_unit(n, dep=False):
                """Keep-warm matmuls into a scratch psum tile; output unread.
                Covers PE idle windows where no real work is dependency-free
                so the p-state never drops and holes close. With dep=True the
                moving operand is the most recent normalize raw tile, which
                pins the junk to run during that normalize chain instead of
                being hoisted into earlier idle slots by the scheduler."""
                ps = ps_s.tile([P, 2, QT_TILE], f32, tag="s", name="ps_junk")
                if dep and LAST_RAW["ap"] is not None:
                    rhs, lhsT = LAST_RAW["ap"][0:HD, :], tri[0:HD, :]
                else:
                    rhs, lhsT = wq_sb[:, 0, :], tri[:]
                for _ in range(n):
                    nc.tensor.matmul(ps[:, 0, 0:rhs.shape[-1]], lhsT, rhs,
                                     start=True, stop=True)

            # ---- attention ----
            def attn_block(qj, bnd0, fillers):
                """bnd0: units emitted at the first sweep's start; they must
                NOT depend on the immediately preceding normalize chain."""
                nk = 4 * qj + 4
                qt_sb = QT_SB[qj]
                ctx_sb = ctxp.tile([P, 2, QT_TILE], DT, tag="ctx",
                                   name=f"ctx{qj}")
                CTX_SB[qj] = ctx_sb
                nfill = len(fillers)
                nsteps = 2 * nk
                # plan[step] = how many filler units to emit after that step.
                # 2 units at the second-sweep boundary hide the normalize
                # chain; the rest spread evenly.
                plan = [0] * nsteps
                avail = nfill
                take = min(2, avail)
                plan[nk] += take
                avail -= take
                mid = [s for s in range(nsteps) if s != nk]
                for r in range(avail):
                    plan[mid[(r * len(mid)) // max(avail, 1)]] += 1
                state = {"step": 0, "done": 0}

                def maybe_fill():
                    if state["step"] == 0:
                        for u in bnd0:
                            u()
                    n = plan[state["step"]]
                    state["step"] += 1
                    for _ in range(n):
                        if state["done"] < nfill:
                            fillers[state["done"]]()
                            state["done"] += 1

                def normalize(ctx_ps_i, row, grp, cl, cw2, stash=False):
                    raw = rawp.tile([HD + 1, QT_TILE], DT, tag="raw",
                                    name="raw")
                    nc.vector.tensor_copy(out=raw[:, 0:cw2],
                                          in_=ctx_ps_i[:, cl:cl + cw2])
                    if stash:
                        LAST_RAW["ap"] = raw
                    den = denp.tile([1, QT_TILE], DT, tag="den", name="den")
                    nc.vector.reciprocal(den[:, 0:cw2],
                                         raw[HD:HD + 1, 0:cw2])
                    rbc = rbcp.tile([HD, QT_TILE], DT, tag="rbc", name="rbc")
                    nc.gpsimd.partition_broadcast(rbc[:, 0:cw2],
                                                  den[:, 0:cw2])
                    nc.vector.tensor_tensor(
                        ctx_sb[row:row + HD, grp, cl:cl + cw2],
                        raw[:HD, 0:cw2], rbc[:, 0:cw2], mult_op,
                    )

                for hp in range(2):
                    ctx_ps = [
                        ps_ctx.tile([HD + 1, QT_TILE], f32, tag="ctx",
                                    name=f"ctx_ps{i}")
                        for i in range(2)
                    ]
                    pending = None

                    def scores_exp(ki):
                        j = ki - 4 * qj
                        col0 = max(0, j) * P
                        sps = ps_s.tile([P, 2, QT_TILE], f32, tag="s",
                                        name="s_ps")
                        for i in range(2):
                            hr = 64 * i
                            nc.tensor.matmul(
                                sps[:, i, col0:],
                                kt[hr:hr + 64, hp, ki * P:(ki + 1) * P],
                                qt_sb[hr:hr + 64, hp, col0:],
                                start=True, stop=True,
                            )
                        et = etp.tile([P, 2, QT_TILE], DT, tag="et", name="et")
                        nc.scalar.activation(
                            et[:, :, col0:], sps[:, :, col0:], Exp, scale=0.125
                        )
                        if j >= 0:
                            for i in range(2):
                                nc.vector.tensor_tensor(
                                    et[:, i, col0:col0 + P],
                                    et[:, i, col0:col0 + P], tri[:], mult_op,
                                )
                        return et, col0

                    def pv1(ki, et, col0, i):
                        nc.tensor.matmul(
                            ctx_ps[i][:, col0:], v[:, ki, 2 * hp + i, :],
                            et[:, i, col0:],
                            start=(ki == 0), stop=(ki == nk - 1),
                        )

                    for ki in range(nk):
                        cur = scores_exp(ki)
                        if pending is not None:
                            pv1(*pending, 0)
                            pv1(*pending, 1)
                        maybe_fill()
                        pending = (ki,) + cur

                    # last PV pair, then immediately evacuate the ctx psum
                    # tiles to SBUF (releases the psum slots in ~1 copy);
                    # normalize runs SBUF->SBUF in bf16 (DVE 4x)
                    raw = [rawp.tile([HD + 1, QT_TILE], DT, tag="raw",
                                     name="raw") for _ in range(2)]
                    den = [denp.tile([1, QT_TILE], DT, tag="den", name="den")
                           for _ in range(2)]
                    rbc = [rbcp.tile([HD, QT_TILE], DT, tag="rbc", name="rbc")
                           for _ in range(2)]
                    pv1(*pending, 0)
                    nc.vector.tensor_copy(out=raw[0][:], in_=ctx_ps[0][:])
                    pv1(*pending, 1)
                    nc.vector.tensor_copy(out=raw[1][:], in_=ctx_ps[1][:])
                    LAST_RAW["ap"] = raw[0]
                    for i in range(2):
                        nc.vector.reciprocal(den[i][:], raw[i][HD:HD + 1, :])
                    for i in range(2):
                        nc.gpsimd.partition_broadcast(rbc[i][:], den[i][:])
                    for i in range(2):
                        nc.vector.tensor_tensor(
                            ctx_sb[64 * i:64 * i + 64, hp, :],
                            raw[i][:HD, :], rbc[i][:], mult_op,
                        )

                while state["done"] < nfill:
                    fillers[state["done"]]()
                    state["done"] += 1

            def attn_fill_lists(qj):
                nxt = proj_units(qj + 1) if qj + 1 < NSJ else []
                prv = oproj_units(qj - 1) if qj > 0 else []
                # out-proj fillers first (ready sooner), DMA-gated
                # projection fillers later in the sweep
                return nxt[:3], prv + nxt[3:]

            def attn_block_q3():
                """Last q-chunk, full-width head-pair sweeps (paired exp).
                In the hp1 sweep, ctx psum columns 0:256 are complete once
                k-block 13 is accumulated (blocks 14/15 only touch columns
                256+), so the first half is normalized inline and its
                out-projection overlaps the rest of the sweep, shrinking the
                kernel tail."""
                qj = NSJ - 1
                nk = 4 * qj + 4
                qt_sb = QT_SB[qj]
                ctx_sb = ctxp.tile([P, 2, QT_TILE], DT, tag="ctx",
                                   name=f"ctx{qj}")
                CTX_SB[qj] = ctx_sb
                prv = oproj_units(qj - 1)
                oq3 = [lambda si=si, dj=dj: oproj_unit(qj, si, dj)
                       for si in (4 * qj, 4 * qj + 1) for dj in range(2)]

                def normalize(ctx_ps_i, row, grp, cl, cw2, stash=True):
                    raw = rawp.tile([HD + 1, QT_TILE], DT, tag="raw",
                                    name="raw")
                    nc.vector.tensor_copy(out=raw[:, 0:cw2],
                                          in_=ctx_ps_i[:, cl:cl + cw2])
                    if stash:
                        LAST_RAW["ap"] = raw
                    den = denp.tile([1, QT_TILE], DT, tag="den", name="den")
                    nc.vector.reciprocal(den[:, 0:cw2],
                                         raw[HD:HD + 1, 0:cw2])
                    rbc = rbcp.tile([HD, QT_TILE], DT, tag="rbc", name="rbc")
                    nc.gpsimd.partition_broadcast(rbc[:, 0:cw2],
                                                  den[:, 0:cw2])
                    nc.vector.tensor_tensor(
                        ctx_sb[row:row + HD, grp, cl:cl + cw2],
                        raw[:HD, 0:cw2], rbc[:, 0:cw2], mult_op,
                    )

                for hp in range(2):
                    ctx_ps = [ps_ctx.tile([HD + 1, QT_TILE], f32, tag="ctx",
                                          name=f"ctx_ps{i}")
                              for i in range(2)]
                    pending = None

                    def scores_exp(ki):
                        j = ki - 4 * qj
                        col0 = max(0, j) * P
                        sps = ps_s.tile([P, 2, QT_TILE], f32, tag="s",
                                        name="s_ps")
                        for i in range(2):
                            hr = 64 * i
                            nc.tensor.matmul(
                                sps[:, i, col0:],
                                kt[hr:hr + 64, hp, ki * P:(ki + 1) * P],
                                qt_sb[hr:hr + 64, hp, col0:],
                                start=True, stop=True,
                            )
                        et = etp.tile([P, 2, QT_TILE], DT, tag="et", name="et")
                        nc.scalar.activation(
                            et[:, :, col0:], sps[:, :, col0:], Exp, scale=0.125
                        )
                        if j >= 0:
                            for i in range(2):
                                nc.vector.tensor_tensor(
                                    et[:, i, col0:col0 + P],
                                    et[:, i, col0:col0 + P], tri[:], mult_op,
                                )
                        return et, col0

                    def pv1(ki, et, col0, i):
                        nc.tensor.matmul(
                            ctx_ps[i][:, col0:], v[:, ki, 2 * hp + i, :],
                            et[:, i, col0:],
                            start=(ki == 0), stop=(ki == nk - 1),
                        )

                    fillers = prv[:5] if hp == 0 else prv[5:]
                    nf = len(fillers)
                    done = 0
                    for ki in range(nk):
                        cur = scores_exp(ki)
                        if pending is not None:
                            pv1(*pending, 0)
                            pv1(*pending, 1)
                            if hp == 1 and pending[0] == nk - 3:
                                # columns 0:256 are final: normalize them now
                                # and unlock the first-half out-projection
                                for i in range(2):
                                    normalize(ctx_ps[i], 64 * i, 1, 0, 256)
                        if ki == 0:
                            junk_unit(3, dep=True)
                        want = ((ki + 1) * nf) // nk
                        while done < want:
                            fillers[done]()
                            done += 1
                        if hp == 1 and ki >= nk - 2 and oq3:
                            oq3.pop(0)()
                        pending = (ki,) + cur

                    pv1(*pending, 0)
                    pv1(*pending, 1)
                    if hp == 0:
                        for i in range(2):
                            normalize(ctx_ps[i], 64 * i, 0, 0, QT_TILE)
                    else:
                        for i in range(2):
                            normalize(ctx_ps[i], 64 * i, 1, 256, 256)
                        for u in oq3:
                            u()

            # ---- emission: pipelined across q-chunks ----
            # prelude: V units first (only need the first half xt chunk + wv)
            u0 = proj_units(0)
            for u in u0[4:] + u0[:4]:
                u()
            for qj in range(NSJ - 1):
                bnd0, fillers = attn_fill_lists(qj)
                attn_block(qj, bnd0, fillers)
            attn_block_q3()
            # bridge the final normalize chain (the out-projection below is
            # the only real work left and strictly depends on it)
            junk_unit(5, dep=True)
            for si in (NSI - 2, NSI - 1):
                for dj in range(2):
                    if si == NSI - 1 and dj == 1:
                        continue
                    oproj_unit(NSJ - 1, si, dj, split_copy=True)
            # split the very last store so the tail DMA is half-length
            oproj_unit(NSJ - 1, NSI - 1, 1, c0=0, cw=256)
            oproj_unit(NSJ - 1, NSI - 1, 1, c0=256, cw=256, on_act=True)

    nc.compile()
    return nc


def _get_nc(mm_mode: str = "bf16"):
    if mm_mode not in _BUILD_CACHE:
        _BUILD_CACHE[mm_mode] = build_bass(mm_mode)
    return _BUILD_CACHE[mm_mode]


def kernel(X, Wq, Wk, Wv, Wo, bo, mm_mode: str = "bf16"):
    from concourse.bass_utils import run_bass_kernel_spmd

    X = np.asarray(X, dtype=np.float32)
    Wq = np.asarray(Wq, dtype=np.float32)
    Wk = np.asarray(Wk, dtype=np.float32)
    Wv = np.asarray(Wv, dtype=np.float32)
    Wo = np.asarray(Wo, dtype=np.float32)
    bo = np.asarray(bo, dtype=np.float32)

    if mm_mode == "bf16":
        import ml_dtypes
        host_dt = np.dtype(ml_dtypes.bfloat16)
    else:
        host_dt = np.dtype(np.float32)

    nc = _get_nc(mm_mode)

    # pre-transpose X per batch: XT [D, S]
    XT = [np.ascontiguousarray(X[b].T).astype(host_dt) for b in range(B)]

    in_maps = []
    for c in range(N_CORES):
        b, g = c // 4, c % 4
        cs = slice(g * DM, (g + 1) * DM)
        in_maps.append({
            "XT": XT[b],
            "Wq": np.ascontiguousarray(Wq[:, cs]).astype(host_dt),
            "Wk": np.ascontiguousarray(Wk[:, cs]).astype(host_dt),
            "Wv": np.ascontiguousarray(Wv[:, cs]).astype(host_dt),
            "Wo": np.ascontiguousarray(Wo[cs, :]).astype(host_dt),
        })

    res = run_bass_kernel_spmd(nc, in_maps, core_ids=list(range(N_CORES)))
    out = np.zeros((B, S, D), dtype=np.float64)
    for c in range(N_CORES):
        out[c // 4] += res.results[c]["Out"].astype(np.float64)
    out += bo.astype(np.float64)
    return out.astype(np.float32)


# revision 54
# speedup vs baseline: 1.0126x; 1.0126x over previous
"""Multi-head causal attention (B=2, S=2048, D=1024, H=16) on 8 TRN2 NeuronCores.

Sharding: batch x head-group tensor parallel. Core c owns batch c//4 and
heads [4*(c%4), 4*(c%4)+4) (a DM=256 model-dim shard). The host
pre-transposes X per batch to XT [D, S] and casts all device inputs to
bf16; each core computes a partial output [S, D] for its batch; the host
sums the 4 partials per batch and adds bo.

Per-core program (all matmul operands bf16, PSUM accumulate f32):
  QT/KT = W^T XT    [256, 2048], m on partitions (2 groups of 128)
  V     = XT^T Wv   [2048, 256] natural (s on partitions), stored per
          head with an appended ones column (softmax denominator trick)
  attention per q-chunk (512) per head pair: scoresT = K Q^T on PE,
  exp on ACT (scale=1/8, both heads of the pair in one activation off a
  2-bank psum tile), causal diagonal-block mask applied multiplicatively
  post-exp on DVE (triangular constant, bf16 4x mode), PV with the ones
  column -> ctxT[64] + denominator row, reciprocal + gpsimd
  partition_broadcast -> normalized ctxT in bf16.
  out[s, :] = ctxT^T Wo (2 dm-chunks accumulated), f16 copy, DMA out.

Emission interleaves projection / out-projection psum groups into the
attention ki-loops (skew-1 scores->PV) so the in-order PE queue never
stalls on the exp->PV dependency.
"""

import numpy as np

B, S, D = 2, 2048, 1024
H_PER_CORE = 4
HD = 64
DM = H_PER_CORE * HD   # 256, per-core model-dim shard
N_CORES = 8
P = 128
QT_TILE = 512          # q free-dim tile in attention
KO = D // P            # 8 contraction chunks for projections
NSI = S // P           # 16 s-chunks of 128
NSJ = S // QT_TILE     # 4 s-chunks of 512

_BUILD_CACHE = {}


def build_bass(mm_mode: str = "bf16"):
    """Build the per-core Bass program. mm_mode in {bf16, fp32r}."""
    import contextlib

    import concourse.tile as tile
    from concourse import bacc, mybir
    from concourse.masks import make_upper_triangular

    f32 = mybir.dt.float32
    f16 = mybir.dt.float16
    DT = mybir.dt.bfloat16 if mm_mode == "bf16" else mybir.dt.float32r
    Exp = mybir.ActivationFunctionType.Exp
    mult_op = mybir.AluOpType.mult

    nc = bacc.Bacc("TRN2", target_bir_lowering=False, debug=False)

    XTd = nc.dram_tensor("XT", [D, S], DT, kind="ExternalInput").ap()
    Wq = nc.dram_tensor("Wq", [D, DM], DT, kind="ExternalInput").ap()
    Wk = nc.dram_tensor("Wk", [D, DM], DT, kind="ExternalInput").ap()
    Wv = nc.dram_tensor("Wv", [D, DM], DT, kind="ExternalInput").ap()
    Wo = nc.dram_tensor("Wo", [DM, D], DT, kind="ExternalInput").ap()
    Out = nc.dram_tensor("Out", [S, D], f16, kind="ExternalOutput").ap()

    lp_ctx = (nc.allow_low_precision(reason="bf16 rounding is intentional")
              if hasattr(nc, "allow_low_precision") else contextlib.nullcontext())
    with lp_ctx, tile.TileContext(nc) as tc:
        with tc.tile_pool(name="consts", bufs=1) as consts, \
             tc.tile_pool(name="wpool", bufs=1) as wpool, \
             tc.tile_pool(name="qt", bufs=2) as qtp, \
             tc.tile_pool(name="et", bufs=6) as etp, \
             tc.tile_pool(name="ctx", bufs=2) as ctxp, \
             tc.tile_pool(name="raw", bufs=3) as rawp, \
             tc.tile_pool(name="den", bufs=4) as denp, \
             tc.tile_pool(name="rbc", bufs=3) as rbcp, \
             tc.tile_pool(name="outp", bufs=8) as outp, \
             tc.tile_pool(name="ps_mm", bufs=2, space="PSUM") as ps_mm, \
             tc.tile_pool(name="ps_s", bufs=2, space="PSUM") as ps_s, \
             tc.tile_pool(name="ps_ctx", bufs=2, space="PSUM") as ps_ctx:

            # ---- constants ----
            # tri[k, q] = 1 where k <= q else 0 (keep-mask for the causal
            # diagonal 128x128 block of scoresT)
            tri = consts.tile([P, P], DT, tag="tri")
            make_upper_triangular(nc, tri[:], val=1.0, diag=True)

            # ---- persistent sbuf tensors ----
            xt = wpool.tile([P, KO, S], DT, tag="xt")
            wq_sb = wpool.tile([P, KO, DM], DT, tag="wq")
            wk_sb = wpool.tile([P, KO, DM], DT, tag="wk")
            wv_sb = wpool.tile([P, KO, DM], DT, tag="wv")
            wo_sb = wpool.tile([P, 2, D], DT, tag="wo")
            kt = wpool.tile([P, 2, S], DT, tag="kt")
            v = wpool.tile([P, NSI, H_PER_CORE, HD + 1], DT, tag="v")

            # warm the Act exp table during the DMA head so the load does
            # not stall the first attention chain
            warm = consts.tile([1, 1], DT, tag="warm")
            nc.scalar.activation(warm[:], tri[0:1, 0:1], Exp, scale=1.0)

            # warm-up matmuls: keep PE busy through the DMA head so the
            # p-state ramp is fully paid on junk work and the first real
            # matmuls run at peak clock
            wps = ps_s.tile([P, 2, QT_TILE], f32, tag="s", name="ps_warm")
            for _ in range(56):
                nc.tensor.matmul(wps[:, 0, 0:P], tri[:], tri[:],
                                 start=True, stop=True)

            # ---- input DMAs (HWDGE), ordered by first use; transfers are
            # effectively serialized on the DMA engines, so order = need ----
            nc.sync.dma_start(
                wv_sb[:, 0:4, :],
                Wv[0:4 * P, :].rearrange("(ko p) m -> p ko m", p=P))
            nc.scalar.dma_start(
                xt[:, :, 0:256],
                XTd[:, 0:256].rearrange("(ko p) s -> p ko s", p=P))
            nc.sync.dma_start(
                wv_sb[:, 4:8, :],
                Wv[4 * P:8 * P, :].rearrange("(ko p) m -> p ko m", p=P))
            nc.scalar.dma_start(
                xt[:, :, 256:QT_TILE],
                XTd[:, 256:QT_TILE].rearrange("(ko p) s -> p ko s", p=P))
            nc.sync.dma_start(wq_sb[:], Wq.rearrange("(ko p) m -> p ko m", p=P))
            nc.sync.dma_start(wk_sb[:], Wk.rearrange("(ko p) m -> p ko m", p=P))
            xt_sl = [slice(qj * QT_TILE, (qj + 1) * QT_TILE)
                     for qj in range(1, NSJ)]
            nc.scalar.dma_start(
                xt[:, :, xt_sl[0]],
                XTd[:, xt_sl[0]].rearrange("(ko p) s -> p ko s", p=P))
            nc.sync.dma_start(wo_sb[:], Wo.rearrange("(g p) n -> p g n", p=P))
            for sl in xt_sl[1:]:
                nc.scalar.dma_start(
                    xt[:, :, sl], XTd[:, sl].rearrange("(ko p) s -> p ko s", p=P)
                )

            # ones columns of V (denominator accumulator rows)
            nc.vector.memset(v[:, :, :, HD:HD + 1], 1.0)

            QT_SB = {}   # qj -> qt tile
            CTX_SB = {}  # qj -> ctx sbuf tile

            # ---- psum-group "units" (interleave fillers) ----
            def qk_unit(qj, g, w_sb, dst_ap):
                ps = ps_mm.tile([P, QT_TILE], f32, tag="mm", name="ps_qk")
                sl = slice(qj * QT_TILE, (qj + 1) * QT_TILE)
                for ko in range(KO):
                    nc.tensor.matmul(
                        ps[:], w_sb[:, ko, g * P:(g + 1) * P], xt[:, ko, sl],
                        start=(ko == 0), stop=(ko == KO - 1),
                    )
                nc.vector.tensor_copy(out=dst_ap, in_=ps[:])

            def v_unit(si):
                ps = ps_mm.tile([P, DM], f32, tag="mm", name="ps_v")
                for ko in range(KO):
                    nc.tensor.matmul(
                        ps[:], xt[:, ko, si * P:(si + 1) * P], wv_sb[:, ko, :],
                        start=(ko == 0), stop=(ko == KO - 1),
                    )
                nc.vector.tensor_copy(
                    out=v[:, si, :, 0:HD],
                    in_=ps[:].rearrange("p (h d) -> p h d", d=HD),
                )

            def proj_units(qj):
                qt_sb = qtp.tile([P, 2, QT_TILE], DT, tag="qt", name=f"qt{qj}")
                QT_SB[qj] = qt_sb
                units = []
                for g in range(2):
                    units.append(lambda g=g: qk_unit(qj, g, wq_sb, qt_sb[:, g, :]))
                for g in range(2):
                    units.append(lambda g=g: qk_unit(
                        qj, g, wk_sb,
                        kt[:, g, qj * QT_TILE:(qj + 1) * QT_TILE]))
                for si in range(4 * qj, 4 * qj + 4):
                    units.append(lambda si=si: v_unit(si))
                return units

            def oproj_unit(qj, si, dj, split_copy=False, c0=0, cw=QT_TILE,
                           on_act=False):
                ps = ps_mm.tile([P, QT_TILE], f32, tag="mm", name="ps_o")
                sc = (si % 4) * P
                n0 = dj * QT_TILE + c0
                for g in range(2):
                    nc.tensor.matmul(
                        ps[:, 0:cw], CTX_SB[qj][:, g, sc:sc + P],
                        wo_sb[:, g, n0:n0 + cw],
                        start=(g == 0), stop=(g == 1),
                    )
                ot = outp.tile([P, QT_TILE], f16, tag="ot", name="ot")
                if split_copy:
                    # tail-only: drain each psum group with both engines at
                    # once so the two mm-slot round trip stays short
                    h = cw // 2
                    nc.vector.tensor_copy(out=ot[:, 0:h], in_=ps[:, 0:h])
                    nc.scalar.copy(out=ot[:, h:cw], in_=ps[:, h:cw])
                elif on_act:
                    nc.scalar.copy(out=ot[:, 0:cw], in_=ps[:, 0:cw])
                else:
                    nc.vector.tensor_copy(out=ot[:, 0:cw], in_=ps[:, 0:cw])
                nc.sync.dma_start(
                    Out[si * P:(si + 1) * P, n0:n0 + cw],
                    ot[:, 0:cw],
                )

            def oproj_units(qj, alternate=False):
                return [
                    lambda si=si, dj=dj: oproj_unit(
                        qj, si, dj, split_copy=alternate)
                    for si in range(4 * qj, 4 * qj + 4) for dj in range(2)
                ]

            LAST_RAW = {"ap": None}

            def junk_unit(n, dep=False):
                """Keep-warm matmuls into a scratch psum tile; output unread.
                Covers PE idle windows where no real work is dependency-free
                so the p-state never drops and holes close. With dep=True the
                moving operand is the most recent normalize raw tile, which
                pins the junk to run during that normalize chain instead of
                being hoisted into earlier idle slots by the scheduler."""
                ps = ps_s.tile([P, 2, QT_TILE], f32, tag="s", name="ps_junk")
                if dep and LAST_RAW["ap"] is not None:
                    rhs, lhsT = LAST_RAW["ap"][0:HD, :], tri[0:HD, :]
                else:
                    rhs, lhsT = wq_sb[:, 0, :], tri[:]
                for _ in range(n):
                    nc.tensor.matmul(ps[:, 0, 0:rhs.shape[-1]], lhsT, rhs,
                                     start=True, stop=True)

            # ---- attention ----
            def attn_block(qj, bnd0, fillers):
                """bnd0: units emitted at the first sweep's start; they must
                NOT depend on the immediately preceding normalize chain."""
                nk = 4 * qj + 4
                qt_sb = QT_SB[qj]
                ctx_sb = ctxp.tile([P, 2, QT_TILE], DT, tag="ctx",
                                   name=f"ctx{qj}")
                CTX_SB[qj] = ctx_sb
                nfill = len(fillers)
                nsteps = 2 * nk
                # plan[step] = how many filler units to emit after that step.
                # 2 units at the second-sweep boundary hide the normalize
                # chain; the rest spread evenly.
                plan = [0] * nsteps
                avail = nfill
                take = min(2, avail)
                plan[nk] += take
                avail -= take
                mid = [s for s in range(nsteps) if s != nk]
                for r in range(avail):
                    plan[mid[(r * len(mid)) // max(avail, 1)]] += 1
                state = {"step": 0, "done": 0}

                def maybe_fill():
                    if state["step"] == 0:
                        for u in bnd0:
                            u()
                    n = plan[state["step"]]
                    state["step"] += 1
                    for _ in range(n):
                        if state["done"] < nfill:
                            fillers[state["done"]]()
                            state["done"] += 1

                def normalize(ctx_ps_i, row, grp, cl, cw2, stash=False):
                    raw = rawp.tile([HD + 1, QT_TILE], DT, tag="raw",
                                    name="raw")
                    nc.vector.tensor_copy(out=raw[:, 0:cw2],
                                          in_=ctx_ps_i[:, cl:cl + cw2])
                    if stash:
                        LAST_RAW["ap"] = raw
                    den = denp.tile([1, QT_TILE], DT, tag="den", name="den")
                    nc.vector.reciprocal(den[:, 0:cw2],
                                         raw[HD:HD + 1, 0:cw2])
                    rbc = rbcp.tile([HD, QT_TILE], DT, tag="rbc", name="rbc")
                    nc.gpsimd.partition_broadcast(rbc[:, 0:cw2],
                                                  den[:, 0:cw2])
                    nc.vector.tensor_tensor(
                        ctx_sb[row:row + HD, grp, cl:cl + cw2],
                        raw[:HD, 0:cw2], rbc[:, 0:cw2], mult_op,
                    )

                for hp in range(2):
                    ctx_ps = [
                        ps_ctx.tile([HD + 1, QT_TILE], f32, tag="ctx",
                                    name=f"ctx_ps{i}")
                        for i in range(2)
                    ]
                    pending = None

                    def scores_exp(ki):
                        j = ki - 4 * qj
                        col0 = max(0, j) * P
                        sps = ps_s.tile([P, 2, QT_TILE], f32, tag="s",
                                        name="s_ps")
                        for i in range(2):
                            hr = 64 * i
                            nc.tensor.matmul(
                                sps[:, i, col0:],
                                kt[hr:hr + 64, hp, ki * P:(ki + 1) * P],
                                qt_sb[hr:hr + 64, hp, col0:],
                                start=True, stop=True,
                            )
                        et = etp.tile([P, 2, QT_TILE], DT, tag="et", name="et")
                        nc.scalar.activation(
                            et[:, :, col0:], sps[:, :, col0:], Exp, scale=0.125
                        )
                        if j >= 0:
                            for i in range(2):
                                nc.vector.tensor_tensor(
                                    et[:, i, col0:col0 + P],
                                    et[:, i, col0:col0 + P], tri[:], mult_op,
                                )
                        return et, col0

                    def pv1(ki, et, col0, i):
                        nc.tensor.matmul(
                            ctx_ps[i][:, col0:], v[:, ki, 2 * hp + i, :],
                            et[:, i, col0:],
                            start=(ki == 0), stop=(ki == nk - 1),
                        )

                    for ki in range(nk):
                        cur = scores_exp(ki)
                        if pending is not None:
                            pv1(*pending, 0)
                            pv1(*pending, 1)
                        maybe_fill()
                        pending = (ki,) + cur

                    # last PV pair, then immediately evacuate the ctx psum
                    # tiles to SBUF (releases the psum slots in ~1 copy);
                    # normalize runs SBUF->SBUF in bf16 (DVE 4x)
                    raw = [rawp.tile([HD + 1, QT_TILE], DT, tag="raw",
                                     name="raw") for _ in range(2)]
                    den = [denp.tile([1, QT_TILE], DT, tag="den", name="den")
                           for _ in range(2)]
                    rbc = [rbcp.tile([HD, QT_TILE], DT, tag="rbc", name="rbc")
                           for _ in range(2)]
                    pv1(*pending, 0)
                    nc.vector.tensor_copy(out=raw[0][:], in_=ctx_ps[0][:])
                    pv1(*pending, 1)
                    nc.vector.tensor_copy(out=raw[1][:], in_=ctx_ps[1][:])
                    LAST_RAW["ap"] = raw[0]
                    for i in range(2):
                        nc.vector.reciprocal(den[i][:], raw[i][HD:HD + 1, :])
                    for i in range(2):
                        nc.gpsimd.partition_broadcast(rbc[i][:], den[i][:])
                    for i in range(2):
                        nc.vector.tensor_tensor(
                            ctx_sb[64 * i:64 * i + 64, hp, :],
                            raw[i][:HD, :], rbc[i][:], mult_op,
                        )

                while state["done"] < nfill:
                    fillers[state["done"]]()
                    state["done"] += 1

            def attn_fill_lists(qj):
                nxt = proj_units(qj + 1) if qj + 1 < NSJ else []
                prv = oproj_units(qj - 1) if qj > 0 else []
                # out-proj fillers first (ready sooner), DMA-gated
                # projection fillers later in the sweep
                return nxt[:3], prv + nxt[3:]

            def attn_block_q3():
                """Last q-chunk, full-width head-pair sweeps (paired exp).
                In the hp1 sweep, ctx psum columns 0:256 are complete once
                k-block 13 is accumulated (blocks 14/15 only touch columns
                256+), so the first half is normalized inline and its
                out-projection overlaps the rest of the sweep, shrinking the
                kernel tail."""
                qj = NSJ - 1
                nk = 4 * qj + 4
                qt_sb = QT_SB[qj]
                ctx_sb = ctxp.tile([P, 2, QT_TILE], DT, tag="ctx",
                                   name=f"ctx{qj}")
                CTX_SB[qj] = ctx_sb
                prv = oproj_units(qj - 1)
                oq3 = [lambda si=si, dj=dj: oproj_unit(qj, si, dj)
                       for si in (4 * qj, 4 * qj + 1) for dj in range(2)]

                def normalize(ctx_ps_i, row, grp, cl, cw2, stash=True):
                    raw = rawp.tile([HD + 1, QT_TILE], DT, tag="raw",
                                    name="raw")
                    nc.vector.tensor_copy(out=raw[:, 0:cw2],
                                          in_=ctx_ps_i[:, cl:cl + cw2])
                    if stash:
                        LAST_RAW["ap"] = raw
                    den = denp.tile([1, QT_TILE], DT, tag="den", name="den")
                    nc.vector.reciprocal(den[:, 0:cw2],
                                         raw[HD:HD + 1, 0:cw2])
                    rbc = rbcp.tile([HD, QT_TILE], DT, tag="rbc", name="rbc")
                    nc.gpsimd.partition_broadcast(rbc[:, 0:cw2],
                                                  den[:, 0:cw2])
                    nc.vector.tensor_tensor(
                        ctx_sb[row:row + HD, grp, cl:cl + cw2],
                        raw[:HD, 0:cw2], rbc[:, 0:cw2], mult_op,
                    )

                for hp in range(2):
                    ctx_ps = [ps_ctx.tile([HD + 1, QT_TILE], f32, tag="ctx",
                                          name=f"ctx_ps{i}")
                              for i in range(2)]
                    pending = None

                    def scores_exp(ki):
                        j = ki - 4 * qj
                        col0 = max(0, j) * P
                        sps = ps_s.tile([P, 2, QT_TILE], f32, tag="s",
                                        name="s_ps")
                        for i in range(2):
                            hr = 64 * i
                            nc.tensor.matmul(
                                sps[:, i, col0:],
                                kt[hr:hr + 64, hp, ki * P:(ki + 1) * P],
                                qt_sb[hr:hr + 64, hp, col0:],
                                start=True, stop=True,
                            )
                        et = etp.tile([P, 2, QT_TILE], DT, tag="et", name="et")
                        nc.scalar.activation(
                            et[:, :, col0:], sps[:, :, col0:], Exp, scale=0.125
                        )
                        if j >= 0:
                            for i in range(2):
                                nc.vector.tensor_tensor(
                                    et[:, i, col0:col0 + P],
                                    et[:, i, col0:col0 + P], tri[:], mult_op,
                                )
                        return et, col0

                    def pv1(ki, et, col0, i):
                        nc.tensor.matmul(
                            ctx_ps[i][:, col0:], v[:, ki, 2 * hp + i, :],
                            et[:, i, col0:],
                            start=(ki == 0), stop=(ki == nk - 1),
                        )

                    fillers = prv[:5] if hp == 0 else prv[5:]
                    nf = len(fillers)
                    done = 0
                    for ki in range(nk):
                        cur = scores_exp(ki)
                        if pending is not None:
                            pv1(*pending, 0)
                            pv1(*pending, 1)
                            if hp == 1 and pending[0] == nk - 3:
                                # columns 0:256 are final: normalize them now
                                # and unlock the first-half out-projection
                                for i in range(2):
                                    normalize(ctx_ps[i], 64 * i, 1, 0, 256)
                        if ki == 0:
                            junk_unit(3, dep=True)
                        want = ((ki + 1) * nf) // nk
                        while done < want:
                            fillers[done]()
                            done += 1
                        if hp == 1 and ki >= nk - 2 and oq3:
                            oq3.pop(0)()
                        pending = (ki,) + cur

                    pv1(*pending, 0)
                    pv1(*pending, 1)
                    if hp == 0:
                        for i in range(2):
                            normalize(ctx_ps[i], 64 * i, 0, 0, QT_TILE)
                    else:
                        for i in range(2):
                            normalize(ctx_ps[i], 64 * i, 1, 256, 256)
                        for u in oq3:
                            u()

            # ---- emission: pipelined across q-chunks ----
            # prelude: V units first (only need the first half xt chunk + wv)
            u0 = proj_units(0)
            for u in u0[4:] + u0[:4]:
                u()
            for qj in range(NSJ - 1):
                bnd0, fillers = attn_fill_lists(qj)
                attn_block(qj, bnd0, fillers)
            attn_block_q3()
            # bridge the final normalize chain (the out-projection below is
            # the only real work left and strictly depends on it)
            junk_unit(7, dep=True)
            for si in (NSI - 2, NSI - 1):
                for dj in range(2):
                    if si == NSI - 1 and dj == 1:
                        continue
                    oproj_unit(NSJ - 1, si, dj, split_copy=True)
            # split the very last store so the tail DMA is half-length
            oproj_unit(NSJ - 1, NSI - 1, 1, c0=0, cw=256)
            oproj_unit(NSJ - 1, NSI - 1, 1, c0=256, cw=256, on_act=True)

    nc.compile()
    return nc


def _get_nc(mm_mode: str = "bf16"):
    if mm_mode not in _BUILD_CACHE:
        _BUILD_CACHE[mm_mode] = build_bass(mm_mode)
    return _BUILD_CACHE[mm_mode]


def kernel(X, Wq, Wk, Wv, Wo, bo, mm_mode: str = "bf16"):
    from concourse.bass_utils import run_bass_kernel_spmd

    X = np.asarray(X, dtype=np.float32)
    Wq = np.asarray(Wq, dtype=np.float32)
    Wk = np.asarray(Wk, dtype=np.float32)
    Wv = np.asarray(Wv, dtype=np.float32)
    Wo = np.asarray(Wo, dtype=np.float32)
    bo = np.asarray(bo, dtype=np.float32)

    if mm_mode == "bf16":
        import ml_dtypes
        host_dt = np.dtype(ml_dtypes.bfloat16)
    else:
        host_dt = np.dtype(np.float32)

    nc = _get_nc(mm_mode)

    # pre-transpose X per batch: XT [D, S]
    XT = [np.ascontiguousarray(X[b].T).astype(host_dt) for b in range(B)]

    in_maps = []
    for c in range(N_CORES):
        b, g = c // 4, c % 4
        cs = slice(g * DM, (g + 1) * DM)
        in_maps.append({
            "XT": XT[b],
            "Wq": np.ascontiguousarray(Wq[:, cs]).astype(host_dt),
            "Wk": np.ascontiguousarray(Wk[:, cs]).astype(host_dt),
            "Wv": np.ascontiguousarray(Wv[:, cs]).astype(host_dt),
            "Wo": np.ascontiguousarray(Wo[cs, :]).astype(host_dt),
        })

    res = run_bass_kernel_spmd(nc, in_maps, core_ids=list(range(N_CORES)))
    out = np.zeros((B, S, D), dtype=np.float64)
    for c in range(N_CORES):
        out[c // 4] += res.results[c]["Out"].astype(np.float64)
    out += bo.astype(np.float64)
    return out.astype(np.float32)
